# revision 57
# baseline (speedup 1.0000x reference)
"""Trainium2 Bass kernel for a causal self-attention block (nanogpt-style).

Full inputs -> full output. 16 heads sharded 2/core across 8 NeuronCores
(tensor-parallel); each core computes its heads' QKV projection, per-head
RMSNorm + RoPE, causal no-max-softmax attention (scores bounded: q,k are
RMS-normalized so |s|<=64), and a partial c_proj over its 128-dim slice of
the residual. Host sums the 8 bf16 partial outputs in f32.

bf16 rewrite of the f32r baseline (650us -> ~297us measured):
  - all matmuls bf16: 1 cyc/row at any N (f32r measured ~2x slower on HW),
    FWL fast weight loads, half the DMA/SBUF traffic, bf16 output partials
  - exp fused across both heads per chunk: one ACTIVATE over a [128,2,N]
    view of a 2-bank PSUM score tile (144 calls instead of 288)
  - PV accumulation keeps the ones-rider (lhsT=[V_h|ones32], M=96) so the
    softmax denominator accumulates free in py rows 64:96; note start=True
    clears the ENTIRE psum bank, so regions sharing a bank must not carry
    independent start flags (this killed a col-packed variant)
  - 1/den via DVE 32x32 stream-transpose: spread q across lanes, divide only
    16 els/lane, transpose back (vs 55us of [1,512] reciprocals); one K=33
    indicator matmul (sel) replicates both heads' 1/den to 128 partitions
  - softmax-normalize fused into the psum->yT eviction multiply
  - deferred finalize: each q-block's rp/yT/c_proj ops are emitted inside
    the NEXT q-block's chunk loop so the den chain overlaps score matmuls
    (the in-order PE queue otherwise stalls ~3us per q-block)
  - y lags scores by a 3-pair software pipeline (pt pool 8 bufs)
"""

import numpy as np

DIM = 1024
NH = 16
HD = 64
SCALE = 0.12
NC_CORES = 8
HPC = NH // NC_CORES  # 2 heads per core


def _build(T=4096):
    import concourse.bass as bass
    import concourse.tile as tile
    from concourse import mybir

    f32 = mybir.dt.float32
    f32r = mybir.dt.float32r
    bf16 = mybir.dt.bfloat16
    AF = mybir.ActivationFunctionType

    NTT = T // 128   # 32 t-tiles of 128
    NQB = T // 512   # 8 q-blocks of 512
    NG = max(1, NTT // 8)     # norm/rope groups of 8 t-tiles
    GT = NTT // NG
    EPS = float(np.finfo(np.float32).eps)

    nc = bass.Bass("TRN2", target_bir_lowering=False, debug=False,
                   num_devices=NC_CORES)

    xT = nc.declare_dram_parameter("xT", [DIM, T], bf16, isOutput=False).ap()
    wT = nc.declare_dram_parameter("wT", [DIM, 384], bf16, isOutput=False).ap()
    ve = nc.declare_dram_parameter("ve", [T, 192], bf16, isOutput=False).ap()
    sel = nc.declare_dram_parameter("sel", [64, 128], bf16, isOutput=False).ap()
    cosn = nc.declare_dram_parameter("cosn", [T, 16], bf16, isOutput=False).ap()
    sinn = nc.declare_dram_parameter("sinn", [T, 16], bf16, isOutput=False).ap()
    tri = nc.declare_dram_parameter("tri", [128, 128], bf16, isOutput=False).ap()
    iden = nc.declare_dram_parameter("iden", [128, 128], bf16, isOutput=False).ap()
    wcT = nc.declare_dram_parameter("wcT", [128, DIM], bf16, isOutput=False).ap()
    outp = nc.declare_dram_parameter("outp", [T, DIM], bf16, isOutput=True).ap()
    import os
    debug = os.environ.get('KDEBUG', '0') == '1'
    if debug:
        dQT = nc.declare_dram_parameter("dQT", [128, T], bf16, isOutput=True).ap()
        dKT = nc.declare_dram_parameter("dKT", [128, T], bf16, isOutput=True).ap()
        dV = nc.declare_dram_parameter("dV", [128, NTT * 192], bf16, isOutput=True).ap()
        dYT = nc.declare_dram_parameter("dYT", [128, T], bf16, isOutput=True).ap()
        dDEN = nc.declare_dram_parameter("dDEN", [64, T], bf16, isOutput=True).ap()
        dPT = nc.declare_dram_parameter("dPT", [128, 1024], bf16, isOutput=True).ap()

    def r(ap):
        return ap.bitcast(f32r)

    with tile.TileContext(nc) as tc:
        with (
            tc.tile_pool(name="consts", bufs=1) as consts,
            tc.tile_pool(name="persist", bufs=1) as persist,
            tc.tile_pool(name="xstream", bufs=3) as xstream,
            tc.tile_pool(name="tmp", bufs=3) as tmp,
            tc.tile_pool(name="pt", bufs=8) as ptpool,
            tc.tile_pool(name="ob", bufs=3) as obpool,
            tc.tile_pool(name="small", bufs=4) as small,
            tc.tile_pool(name="sc", bufs=2, space="PSUM") as psc,      # 2x2 banks
            tc.tile_pool(name="py", bufs=3, space="PSUM") as pyp,      # 3 banks
            tc.tile_pool(name="dn", bufs=1, space="PSUM") as pdn,      # 1 bank
        ):
            # ---- constants ----
            tri_sb = consts.tile([128, 128], bf16, tag="tri")
            nc.sync.dma_start(tri_sb[:, :], tri)
            id_sb = consts.tile([128, 128], bf16, tag="iden")
            nc.sync.dma_start(id_sb[:, :], iden)
            cos_sb = consts.tile([128, NTT, 16], bf16, tag="cos")
            nc.sync.dma_start(cos_sb[:, :, :],
                              cosn.rearrange("(tt p) i -> p tt i", p=128))
            sin_sb = consts.tile([128, NTT, 16], bf16, tag="sin")
            nc.sync.dma_start(sin_sb[:, :, :],
                              sinn.rearrange("(tt p) i -> p tt i", p=128))
            w_sb = consts.tile([128, 8, 384], bf16, tag="w")
            nc.sync.dma_start(w_sb[:, :, :],
                              wT.rearrange("(dc p) e -> p dc e", p=128))
            wc_sb = consts.tile([128, DIM], bf16, tag="wc")
            nc.sync.dma_start(wc_sb[:, :], wcT)
            eps_sb = consts.tile([128, 1], f32, tag="eps")
            nc.gpsimd.memset(eps_sb[:, :], EPS)
            sel_sb = consts.tile([64, 128], bf16, tag="sel")
            nc.sync.dma_start(sel_sb[:, :], sel)

            # den staging: per head a [32,512] scratch for the stream-transpose
            # reciprocal trick; den2 rows {0,32} end up holding 1/den.

            q_nat = persist.tile([128, NTT, 128], bf16, tag="qnat")
            k_nat = persist.tile([128, NTT, 128], bf16, tag="knat")
            # rider layout: v0 | ones | v1 | ones (denominator rides as
            # redundant ones columns in the M=96 PV matmul)
            v_sb = persist.tile([128, NTT, 192], bf16, tag="v")
            QT = persist.tile([128, T], bf16, tag="QT")
            KT = persist.tile([128, T], bf16, tag="KT")
            yT = persist.tile([128, T], bf16, tag="yT")

            # v preloaded with lambda1 * ve slice (host-scaled)
            nc.sync.dma_start(v_sb[:, :, :],
                              ve.rearrange("(tt p) d -> p tt d", p=128))

            # ---- phases 1-3, pipelined per tile-group of GT=8 ----
            # emit group g's QKV (PE-heavy), then its norm/rope (DVE), then
            # the PREVIOUS group's transposes: the transposes' psum->QT casts
            # then drain while the next group's QKV matmuls keep the PE busy,
            # instead of crawling at DVE-cast rate after all of phase 2.
            def emit_p1(tt):
                xt = xstream.tile([128, 8, 128], bf16, tag="xt",
                                  name=f"xt{tt}")
                nc.sync.dma_start(
                    xt[:, :, :],
                    xT[:, 128 * tt:128 * tt + 128]
                    .rearrange("(dc p) t -> p dc t", p=128))
                ps = psc.tile([128, 1024], f32, tag="sc", name=f"ps1_{tt}")
                for dc in range(8):
                    nc.tensor.matmul(ps[:, 0:384], xt[:, dc, :], w_sb[:, dc, :],
                                     start=(dc == 0), stop=(dc == 7))
                nc.vector.tensor_copy(q_nat[:, tt, :], ps[:, 0:128])
                nc.vector.tensor_copy(k_nat[:, tt, :], ps[:, 128:256])
                nc.vector.tensor_add(v_sb[:, tt, 0:64], ps[:, 256:320],
                                     v_sb[:, tt, 0:64])
                nc.vector.tensor_add(v_sb[:, tt, 96:160], ps[:, 320:384],
                                     v_sb[:, tt, 96:160])

            def emit_p3(tts):
                for tt in tts:
                    for ni, (nat, dstT) in enumerate(((q_nat, QT),
                                                      (k_nat, KT))):
                        pt_ps = pyp.tile([128, 128], bf16, tag="py",
                                         name=f"tp{tt}_{ni}")
                        nc.tensor.transpose(pt_ps[:, :], nat[:, tt, :],
                                            id_sb[:, :])
                        nc.vector.tensor_copy(
                            dstT[:, 128 * tt:128 * tt + 128], pt_ps[:, :])

            for g in range(NG):
                gsl = slice(GT * g, GT * g + GT)
                for tt in range(GT * g, GT * g + GT):
                    emit_p1(tt)
                for nat in (q_nat, k_nat):
                    xg = nat[:, gsl, :]                       # [128, GT, 128]
                    xg4 = nat[:, gsl, :].rearrange("p a (h d) -> p a h d", h=2)
                    sq = tmp.tile([128, GT * 128], bf16, tag="sq")
                    nc.vector.tensor_mul(sq[:, :], xg, xg)
                    ssum = small.tile([128, GT, 2], f32, tag="ssum")
                    nc.vector.reduce_sum(
                        ssum[:, :, :],
                        sq[:, :].rearrange("p (a h d) -> p a h d", a=GT, h=2),
                        axis=mybir.AxisListType.X)
                    sstd = small.tile([128, GT, 2], f32, tag="sstd")
                    nc.scalar.activation(sstd[:, :, :], ssum[:, :, :],
                                         AF.Sqrt, bias=eps_sb[:, :],
                                         scale=1.0 / HD)
                    rinv = small.tile([128, GT, 2], f32, tag="rinv")
                    nc.vector.reciprocal(rinv[:, :, :], sstd[:, :, :])
                    rinvb = small.tile([128, GT, 2], bf16, tag="rinvb")
                    nc.vector.tensor_copy(rinvb[:, :, :], rinv[:, :, :])
                    nc.vector.tensor_mul(
                        xg4, xg4,
                        rinvb[:, :, :].broadcast_to((128, GT, 2, HD)))
                    # rope on pairs (d, d+32), d in [0,16)
                    x1 = nat[:, gsl, :].rearrange("p a (h d) -> p a h d", h=2)[:, :, :, 0:16]
                    x2 = nat[:, gsl, :].rearrange("p a (h d) -> p a h d", h=2)[:, :, :, 32:48]
                    cg = (cos_sb[:, gsl, :].rearrange("p a i -> p a () i")
                          .broadcast_to((128, GT, 2, 16)))
                    sg = (sin_sb[:, gsl, :].rearrange("p a i -> p a () i")
                          .broadcast_to((128, GT, 2, 16)))
                    t1 = tmp.tile([128, GT, 2, 16], bf16, tag="t1")
                    t2 = tmp.tile([128, GT, 2, 16], bf16, tag="t2")
                    t3 = tmp.tile([128, GT, 2, 16], bf16, tag="t3")
                    t4 = tmp.tile([128, GT, 2, 16], bf16, tag="t4")
                    nc.vector.tensor_mul(t1[:, :, :, :], x1, cg)
                    nc.vector.tensor_mul(t2[:, :, :, :], x2, sg)
                    nc.vector.tensor_mul(t3[:, :, :, :], x1, sg)
                    nc.vector.tensor_mul(t4[:, :, :, :], x2, cg)
                    nc.vector.tensor_add(x1, t1[:, :, :, :], t2[:, :, :, :])
                    nc.vector.tensor_sub(x2, t4[:, :, :, :], t3[:, :, :, :])
                if g >= 1:
                    emit_p3(range(GT * (g - 1), GT * g))
            emit_p3(range(GT * (NG - 1), GT * NG))

            # ---- phase 4: causal attention per q-block of 512 ----
            # deferred finalize of the previous q-block, split into small
            # closures drained one per chunk-pair so the in-order PE queue
            # never piles up behind the DVE den chain or c_proj evictions
            pending = []

            def flush_one():
                if pending:
                    pending.pop(0)()

            def flush_all():
                while pending:
                    pending.pop(0)()

            for qb in range(NQB):
                q0 = 512 * qb
                ntc = 4 * (qb + 1)
                py = [pyp.tile([128, 512], f32, tag="py", name=f"py{qb}_{h}")
                      for h in range(2)]
                stage = []  # software pipeline: y lags scores by three pairs
                for pi in range(ntc // 2):
                    if pi == 1:
                        flush_all()
                    pts = []
                    for c in (2 * pi, 2 * pi + 1):
                        p = c - 4 * qb
                        ql = max(0, 128 * p)
                        ts = slice(128 * c, 128 * c + 128)
                        s = psc.tile([128, 1024], f32, tag="sc",
                                     name=f"s{qb}_{c}")
                        for h in range(2):
                            hp = slice(64 * h, 64 * h + 64)
                            nc.tensor.matmul(
                                s[:, 512 * h + ql:512 * h + 512],
                                KT[hp, ts], QT[hp, q0 + ql:q0 + 512],
                                start=True, stop=True,
                                tile_position=(64 * h, 0))
                        pt = ptpool.tile([128, 2, 512], bf16, tag="pt",
                                         name=f"ptile{qb}_{c}")
                        nc.scalar.activation(
                            pt[:, :, ql:512],
                            s[:, :].rearrange("p (h n) -> p h n", h=2)[:, :, ql:512],
                            AF.Exp, scale=SCALE)
                        if p >= 0:
                            nc.vector.tensor_mul(
                                pt[:, :, ql:ql + 128], pt[:, :, ql:ql + 128],
                                tri_sb[:, :].rearrange("p x -> p () x")
                                .broadcast_to((128, 2, 128)))
                        if debug and qb == 1 and c == 2:
                            nc.sync.dma_start(
                                dPT[:, :],
                                pt[:, :, :].rearrange("p h n -> p (h n)"))
                        pts.append((c, ql, pt))
                    stage.append(pts)
                    if len(stage) == 4:
                        _emit_y(nc, v_sb, py, stage.pop(0), ntc)
                while stage:
                    _emit_y(nc, v_sb, py, stage.pop(0), ntc)

                # ---- denominators -> 1/den (cheap path) ----
                # dens ride in py[h] rows 64:96 (32 replicated rows). Stream-
                # transpose the [32,512] rider block so the q values spread
                # across lanes, reciprocal only lane-column 0 (16 els/lane
                # instead of 512), transpose back: den2 row 32h = 1/den_h.
                den2 = small.tile([64, 512], f32, tag="den2", name=f"d2{qb}")
                for h in range(2):
                    xs = small.tile([32, 512], f32, tag="xs",
                                    name=f"xs{qb}_{h}")
                    nc.vector.transpose(xs[0:32, :], py[h][64:96, :])
                    xv = xs[0:32, :].rearrange("p (j c) -> p j c", c=32)
                    nc.vector.reciprocal(xv[:, :, 0:1], xv[:, :, 0:1])
                    nc.vector.transpose(den2[32 * h:32 * h + 32, :],
                                        xs[0:32, :])
                rdenb = small.tile([64, 512], bf16, tag="rdenb",
                                   name=f"rb{qb}")
                nc.vector.tensor_copy(rdenb[0:33, :], den2[0:33, :])
                if debug:
                    nc.sync.dma_start(dDEN[:, q0:q0 + 512], rdenb[0:64, :])

                def fin_norm(qb=qb, q0=q0, py=py, rdenb=rdenb):
                    rp = pdn.tile([128, 512], f32, tag="dn", name=f"rp{qb}")
                    nc.tensor.matmul(rp[:, :], sel_sb[0:33, :],
                                     rdenb[0:33, :], start=True, stop=True)
                    rrs = tmp.tile([128, 512], bf16, tag="rrs",
                                   name=f"rrs{qb}")
                    nc.vector.tensor_copy(rrs[:, :], rp[:, :])
                    for h in range(2):
                        nc.vector.tensor_mul(
                            yT[64 * h:64 * h + 64, q0:q0 + 512],
                            py[h][0:64, :], rrs[64 * h:64 * h + 64, :])

                def mk_cproj(tt):
                    def f():
                        ts2 = slice(128 * tt, 128 * tt + 128)
                        for half in range(2):
                            po = pdn.tile([128, 512], f32, tag="dn",
                                          name=f"po{tt}_{half}")
                            nc.tensor.matmul(
                                po[:, :], yT[:, ts2],
                                wc_sb[:, 512 * half:512 * half + 512],
                                start=True, stop=True)
                            ob = obpool.tile([128, 512], bf16, tag="ob",
                                             name=f"ob{tt}_{half}")
                            nc.vector.tensor_copy(ob[:, :], po[:, :])
                            nc.sync.dma_start(
                                outp[ts2, 512 * half:512 * half + 512],
                                ob[:, :])
                    return f

                flush_all()
                pending.append(fin_norm)
                for tt in range(4 * qb, 4 * qb + 4):
                    pending.append(mk_cproj(tt))
            flush_all()
            if debug:
                nc.sync.dma_start(dQT[:, :], QT[:, :])
                nc.sync.dma_start(dKT[:, :], KT[:, :])
                nc.sync.dma_start(
                    dV[:, :], v_sb[:, :, :].rearrange("p tt d -> p (tt d)"))
                nc.sync.dma_start(dYT[:, :], yT[:, :])
    _cap_matmul_waits(nc)
    return nc


def _emit_y(nc, v_sb, py, pts, ntc):
    """PV accumulation with lhsT = [V_h(64) | ones(32)]: M=96, the ones
    columns accumulate the softmax denominator for free (rows 64:96)."""
    for (c, ql, pt) in pts:
        for h in range(2):
            nc.tensor.matmul(py[h][0:96, ql:512],
                             v_sb[:, c, 96 * h:96 * h + 96],
                             pt[:, h, ql:512],
                             start=(c == 0), stop=(c == ntc - 1))


def _cap_matmul_waits(nc, limit=1):
    """walrus supports few (often one) sync-wait slots per lowered
    instruction; move excess waits onto same-engine nops inserted just
    before, so the sequencer blocks identically but each instruction
    carries at most `limit` waits."""
    import bass_rust
    from concourse import mybir

    eng = {
        mybir.EngineType.PE: nc.tensor,
        mybir.EngineType.DVE: nc.vector,
        mybir.EngineType.Activation: nc.scalar,
        mybir.EngineType.Pool: nc.gpsimd,
        mybir.EngineType.SP: nc.sync,
    }

    def make_nop(e):
        eng[e].nop()
        fn = nc.m.functions[0]
        for obb in fn.blocks:
            if (obb.instructions
                    and type(obb.instructions[-1]).__name__ == 'InstNoOp'):
                return obb.instructions.pop()
        raise AssertionError('nop not found')

    fn = nc.m.functions[0]
    for bb in fn.blocks:
        il = bb.instructions
        i = 0
        while i < len(il):
            inst = il[i]
            si = inst.sync_info
            if (si and si.on_wait and len(si.on_wait) > limit
                    and inst.engine in eng
                    and type(inst).__name__ != 'InstNoOp'):
                waits = list(si.on_wait)
                keep, excess = waits[-limit:], waits[:-limit]
                for w in excess:
                    nop = make_nop(inst.engine)
                    nop.sync_info = bass_rust.SyncInfo(on_wait=[w],
                                                       on_update=[])
                    il.insert(i, nop)
                    i += 1
                inst.sync_info = bass_rust.SyncInfo(
                    on_wait=keep, on_update=list(si.on_update))
            i += 1


def _host_prep(x, ve, qkv_w, lambdas, c_proj_w, T):
    import ml_dtypes
    bf = ml_dtypes.bfloat16
    xT = np.ascontiguousarray(x.reshape(T, DIM).T.astype(bf))
    af = (1.0 / 1024.0) ** np.linspace(0.0, 1.0, HD // 4, dtype=np.float32)
    theta = np.arange(T, dtype=np.float32)[:, None] * af[None, :]
    cos = np.ascontiguousarray(np.cos(theta).astype(bf))
    sin = np.ascontiguousarray(np.sin(theta).astype(bf))
    tri = np.ascontiguousarray(np.triu(np.ones((128, 128), np.float32)).astype(bf))
    iden = np.ascontiguousarray(np.eye(128, dtype=np.float32).astype(bf))
    lam = np.asarray(lambdas, np.float32)
    vef = ve.reshape(T, DIM)
    in_maps = []
    for c in range(NC_CORES):
        sl = slice(128 * c, 128 * c + 128)
        wq = qkv_w[0][sl]
        wk = qkv_w[1][sl]
        wv = qkv_w[2][sl] * lam[0]
        wTl = np.ascontiguousarray(np.concatenate([wq, wk, wv], 0).T.astype(bf))
        ve_l = np.ones((T, 192), np.float32)
        ve_l[:, 0:64] = vef[:, sl.start:sl.start + 64] * lam[1]
        ve_l[:, 96:160] = vef[:, sl.start + 64:sl.stop] * lam[1]
        ve_l = np.ascontiguousarray(ve_l.astype(bf))
        wcTl = np.ascontiguousarray(c_proj_w[:, sl].T.astype(bf))
        selm = np.zeros((64, 128), np.float32)
        selm[0, 0:64] = 1.0
        selm[32, 64:128] = 1.0
        in_maps.append(dict(xT=xT, wT=wTl, ve=ve_l, cosn=cos, sinn=sin,
                            tri=tri, iden=iden, wcT=wcTl,
                            sel=np.ascontiguousarray(selm.astype(bf))))
    return in_maps


LAST_RESULTS = None


def kernel(x, ve, qkv_w, lambdas, c_proj_w):
    import sys
    if '/opt/trn_rl_repo' not in sys.path:
        sys.path.insert(0, '/opt/trn_rl_repo')
    from concourse.bass_utils import run_bass_kernel_spmd

    x = np.asarray(x, np.float32)
    T = x.shape[1]
    in_maps = _host_prep(np.asarray(x, np.float32), np.asarray(ve, np.float32),
                         np.asarray(qkv_w, np.float32),
                         np.asarray(lambdas, np.float32),
                         np.asarray(c_proj_w, np.float32), T)
    nc = _build(T)
    res = run_bass_kernel_spmd(nc, in_maps, core_ids=list(range(NC_CORES)))
    global LAST_RESULTS
    LAST_RESULTS = res
    out = np.zeros((T, DIM), np.float32)
    for rmap in res.results:
        out += rmap["outp"].astype(np.float32)
    return out.reshape(1, T, DIM)


# revision 58
# speedup vs baseline: 1.0102x; 1.0102x over previous
"""Trainium2 Bass kernel for a causal self-attention block (nanogpt-style).

Full inputs -> full output. 16 heads sharded 2/core across 8 NeuronCores
(tensor-parallel); each core computes its heads' QKV projection, per-head
RMSNorm + RoPE, causal no-max-softmax attention (scores bounded: q,k are
RMS-normalized so |s|<=64), and a partial c_proj over its 128-dim slice of
the residual. Host sums the 8 bf16 partial outputs in f32.

bf16 rewrite of the f32r baseline (650us -> ~297us measured):
  - all matmuls bf16: 1 cyc/row at any N (f32r measured ~2x slower on HW),
    FWL fast weight loads, half the DMA/SBUF traffic, bf16 output partials
  - exp fused across both heads per chunk: one ACTIVATE over a [128,2,N]
    view of a 2-bank PSUM score tile (144 calls instead of 288)
  - PV accumulation keeps the ones-rider (lhsT=[V_h|ones32], M=96) so the
    softmax denominator accumulates free in py rows 64:96; note start=True
    clears the ENTIRE psum bank, so regions sharing a bank must not carry
    independent start flags (this killed a col-packed variant)
  - 1/den via DVE 32x32 stream-transpose: spread q across lanes, divide only
    16 els/lane, transpose back (vs 55us of [1,512] reciprocals); one K=33
    indicator matmul (sel) replicates both heads' 1/den to 128 partitions
  - softmax-normalize fused into the psum->yT eviction multiply
  - deferred finalize: each q-block's rp/yT/c_proj ops are emitted inside
    the NEXT q-block's chunk loop so the den chain overlaps score matmuls
    (the in-order PE queue otherwise stalls ~3us per q-block)
  - y lags scores by a 3-pair software pipeline (pt pool 8 bufs)
"""

import numpy as np

DIM = 1024
NH = 16
HD = 64
SCALE = 0.12
NC_CORES = 8
HPC = NH // NC_CORES  # 2 heads per core


def _build(T=4096):
    import concourse.bass as bass
    import concourse.tile as tile
    from concourse import mybir

    f32 = mybir.dt.float32
    f32r = mybir.dt.float32r
    bf16 = mybir.dt.bfloat16
    AF = mybir.ActivationFunctionType

    NTT = T // 128   # 32 t-tiles of 128
    NQB = T // 512   # 8 q-blocks of 512
    NG = max(1, NTT // 8)     # norm/rope groups of 8 t-tiles
    GT = NTT // NG
    EPS = float(np.finfo(np.float32).eps)

    nc = bass.Bass("TRN2", target_bir_lowering=False, debug=False,
                   num_devices=NC_CORES)

    xT = nc.declare_dram_parameter("xT", [DIM, T], bf16, isOutput=False).ap()
    wT = nc.declare_dram_parameter("wT", [DIM, 384], bf16, isOutput=False).ap()
    ve = nc.declare_dram_parameter("ve", [T, 192], bf16, isOutput=False).ap()
    sel = nc.declare_dram_parameter("sel", [64, 128], bf16, isOutput=False).ap()
    cosn = nc.declare_dram_parameter("cosn", [T, 16], bf16, isOutput=False).ap()
    sinn = nc.declare_dram_parameter("sinn", [T, 16], bf16, isOutput=False).ap()
    tri = nc.declare_dram_parameter("tri", [128, 128], bf16, isOutput=False).ap()
    iden = nc.declare_dram_parameter("iden", [128, 128], bf16, isOutput=False).ap()
    wcT = nc.declare_dram_parameter("wcT", [128, DIM], bf16, isOutput=False).ap()
    outp = nc.declare_dram_parameter("outp", [T, DIM], bf16, isOutput=True).ap()
    import os
    debug = os.environ.get('KDEBUG', '0') == '1'
    if debug:
        dQT = nc.declare_dram_parameter("dQT", [128, T], bf16, isOutput=True).ap()
        dKT = nc.declare_dram_parameter("dKT", [128, T], bf16, isOutput=True).ap()
        dV = nc.declare_dram_parameter("dV", [128, NTT * 192], bf16, isOutput=True).ap()
        dYT = nc.declare_dram_parameter("dYT", [128, T], bf16, isOutput=True).ap()
        dDEN = nc.declare_dram_parameter("dDEN", [64, T], bf16, isOutput=True).ap()
        dPT = nc.declare_dram_parameter("dPT", [128, 1024], bf16, isOutput=True).ap()

    def r(ap):
        return ap.bitcast(f32r)

    with tile.TileContext(nc) as tc:
        with (
            tc.tile_pool(name="consts", bufs=1) as consts,
            tc.tile_pool(name="persist", bufs=1) as persist,
            tc.tile_pool(name="xstream", bufs=3) as xstream,
            tc.tile_pool(name="tmp", bufs=3) as tmp,
            tc.tile_pool(name="pt", bufs=8) as ptpool,
            tc.tile_pool(name="ob", bufs=3) as obpool,
            tc.tile_pool(name="small", bufs=4) as small,
            tc.tile_pool(name="sc", bufs=2, space="PSUM") as psc,      # 2x2 banks
            tc.tile_pool(name="py", bufs=3, space="PSUM") as pyp,      # 3 banks
            tc.tile_pool(name="dn", bufs=1, space="PSUM") as pdn,      # 1 bank
        ):
            # ---- constants ----
            tri_sb = consts.tile([128, 128], bf16, tag="tri")
            nc.sync.dma_start(tri_sb[:, :], tri)
            id_sb = consts.tile([128, 128], bf16, tag="iden")
            nc.sync.dma_start(id_sb[:, :], iden)
            cos_sb = consts.tile([128, NTT, 16], bf16, tag="cos")
            nc.sync.dma_start(cos_sb[:, :, :],
                              cosn.rearrange("(tt p) i -> p tt i", p=128))
            sin_sb = consts.tile([128, NTT, 16], bf16, tag="sin")
            nc.sync.dma_start(sin_sb[:, :, :],
                              sinn.rearrange("(tt p) i -> p tt i", p=128))
            w_sb = consts.tile([128, 8, 384], bf16, tag="w")
            nc.sync.dma_start(w_sb[:, :, :],
                              wT.rearrange("(dc p) e -> p dc e", p=128))
            wc_sb = consts.tile([128, DIM], bf16, tag="wc")
            nc.sync.dma_start(wc_sb[:, :], wcT)
            eps_sb = consts.tile([128, 1], f32, tag="eps")
            nc.gpsimd.memset(eps_sb[:, :], EPS)
            sel_sb = consts.tile([64, 128], bf16, tag="sel")
            nc.sync.dma_start(sel_sb[:, :], sel)

            # den staging: per head a [32,512] scratch for the stream-transpose
            # reciprocal trick; den2 rows {0,32} end up holding 1/den.

            q_nat = persist.tile([128, NTT, 128], bf16, tag="qnat")
            k_nat = persist.tile([128, NTT, 128], bf16, tag="knat")
            # rider layout: v0 | ones | v1 | ones (denominator rides as
            # redundant ones columns in the M=96 PV matmul)
            v_sb = persist.tile([128, NTT, 192], bf16, tag="v")
            QT = persist.tile([128, T], bf16, tag="QT")
            KT = persist.tile([128, T], bf16, tag="KT")
            yT = persist.tile([128, T], bf16, tag="yT")

            # v preloaded with lambda1 * ve slice (host-scaled)
            nc.sync.dma_start(v_sb[:, :, :],
                              ve.rearrange("(tt p) d -> p tt d", p=128))

            # ---- phases 1-3, pipelined per tile-group of GT=8 ----
            # emit group g's QKV (PE-heavy), then its norm/rope (DVE), then
            # the PREVIOUS group's transposes: the transposes' psum->QT casts
            # then drain while the next group's QKV matmuls keep the PE busy,
            # instead of crawling at DVE-cast rate after all of phase 2.
            def emit_p1(tt):
                xt = xstream.tile([128, 8, 128], bf16, tag="xt",
                                  name=f"xt{tt}")
                nc.sync.dma_start(
                    xt[:, :, :],
                    xT[:, 128 * tt:128 * tt + 128]
                    .rearrange("(dc p) t -> p dc t", p=128))
                ps = psc.tile([128, 1024], f32, tag="sc", name=f"ps1_{tt}")
                for dc in range(8):
                    nc.tensor.matmul(ps[:, 0:384], xt[:, dc, :], w_sb[:, dc, :],
                                     start=(dc == 0), stop=(dc == 7))
                nc.vector.tensor_copy(q_nat[:, tt, :], ps[:, 0:128])
                nc.vector.tensor_copy(k_nat[:, tt, :], ps[:, 128:256])
                nc.vector.tensor_add(v_sb[:, tt, 0:64], ps[:, 256:320],
                                     v_sb[:, tt, 0:64])
                nc.vector.tensor_add(v_sb[:, tt, 96:160], ps[:, 320:384],
                                     v_sb[:, tt, 96:160])

            def emit_p3(tts):
                for tt in tts:
                    for ni, (nat, dstT) in enumerate(((q_nat, QT),
                                                      (k_nat, KT))):
                        pt_ps = pyp.tile([128, 128], bf16, tag="py",
                                         name=f"tp{tt}_{ni}")
                        nc.tensor.transpose(pt_ps[:, :], nat[:, tt, :],
                                            id_sb[:, :])
                        nc.vector.tensor_copy(
                            dstT[:, 128 * tt:128 * tt + 128], pt_ps[:, :])

            for g in range(NG):
                gsl = slice(GT * g, GT * g + GT)
                for tt in range(GT * g, GT * g + GT):
                    emit_p1(tt)
                if g >= 1:
                    emit_p3(range(GT * (g - 1), GT * g))
                for nat in (q_nat, k_nat):
                    xg = nat[:, gsl, :]                       # [128, GT, 128]
                    xg4 = nat[:, gsl, :].rearrange("p a (h d) -> p a h d", h=2)
                    sq = tmp.tile([128, GT * 128], bf16, tag="sq")
                    nc.vector.tensor_mul(sq[:, :], xg, xg)
                    ssum = small.tile([128, GT, 2], f32, tag="ssum")
                    nc.vector.reduce_sum(
                        ssum[:, :, :],
                        sq[:, :].rearrange("p (a h d) -> p a h d", a=GT, h=2),
                        axis=mybir.AxisListType.X)
                    sstd = small.tile([128, GT, 2], f32, tag="sstd")
                    nc.scalar.activation(sstd[:, :, :], ssum[:, :, :],
                                         AF.Sqrt, bias=eps_sb[:, :],
                                         scale=1.0 / HD)
                    rinv = small.tile([128, GT, 2], f32, tag="rinv")
                    nc.vector.reciprocal(rinv[:, :, :], sstd[:, :, :])
                    rinvb = small.tile([128, GT, 2], bf16, tag="rinvb")
                    nc.vector.tensor_copy(rinvb[:, :, :], rinv[:, :, :])
                    nc.vector.tensor_mul(
                        xg4, xg4,
                        rinvb[:, :, :].broadcast_to((128, GT, 2, HD)))
                    # rope on pairs (d, d+32), d in [0,16)
                    x1 = nat[:, gsl, :].rearrange("p a (h d) -> p a h d", h=2)[:, :, :, 0:16]
                    x2 = nat[:, gsl, :].rearrange("p a (h d) -> p a h d", h=2)[:, :, :, 32:48]
                    cg = (cos_sb[:, gsl, :].rearrange("p a i -> p a () i")
                          .broadcast_to((128, GT, 2, 16)))
                    sg = (sin_sb[:, gsl, :].rearrange("p a i -> p a () i")
                          .broadcast_to((128, GT, 2, 16)))
                    t1 = tmp.tile([128, GT, 2, 16], bf16, tag="t1")
                    t2 = tmp.tile([128, GT, 2, 16], bf16, tag="t2")
                    t3 = tmp.tile([128, GT, 2, 16], bf16, tag="t3")
                    t4 = tmp.tile([128, GT, 2, 16], bf16, tag="t4")
                    nc.vector.tensor_mul(t1[:, :, :, :], x1, cg)
                    nc.vector.tensor_mul(t2[:, :, :, :], x2, sg)
                    nc.vector.tensor_mul(t3[:, :, :, :], x1, sg)
                    nc.vector.tensor_mul(t4[:, :, :, :], x2, cg)
                    nc.vector.tensor_add(x1, t1[:, :, :, :], t2[:, :, :, :])
                    nc.vector.tensor_sub(x2, t4[:, :, :, :], t3[:, :, :, :])
            emit_p3(range(GT * (NG - 1), GT * NG))

            # ---- phase 4: causal attention per q-block of 512 ----
            # deferred finalize of the previous q-block, split into small
            # closures drained one per chunk-pair so the in-order PE queue
            # never piles up behind the DVE den chain or c_proj evictions
            pending = []

            def flush_one():
                if pending:
                    pending.pop(0)()

            def flush_all():
                while pending:
                    pending.pop(0)()

            for qb in range(NQB):
                q0 = 512 * qb
                ntc = 4 * (qb + 1)
                py = [pyp.tile([128, 512], f32, tag="py", name=f"py{qb}_{h}")
                      for h in range(2)]
                stage = []  # software pipeline: y lags scores by three pairs
                for pi in range(ntc // 2):
                    if pi == 1:
                        flush_all()
                    pts = []
                    for c in (2 * pi, 2 * pi + 1):
                        p = c - 4 * qb
                        ql = max(0, 128 * p)
                        ts = slice(128 * c, 128 * c + 128)
                        s = psc.tile([128, 1024], f32, tag="sc",
                                     name=f"s{qb}_{c}")
                        for h in range(2):
                            hp = slice(64 * h, 64 * h + 64)
                            nc.tensor.matmul(
                                s[:, 512 * h + ql:512 * h + 512],
                                KT[hp, ts], QT[hp, q0 + ql:q0 + 512],
                                start=True, stop=True,
                                tile_position=(64 * h, 0))
                        pt = ptpool.tile([128, 2, 512], bf16, tag="pt",
                                         name=f"ptile{qb}_{c}")
                        nc.scalar.activation(
                            pt[:, :, ql:512],
                            s[:, :].rearrange("p (h n) -> p h n", h=2)[:, :, ql:512],
                            AF.Exp, scale=SCALE)
                        if p >= 0:
                            nc.vector.tensor_mul(
                                pt[:, :, ql:ql + 128], pt[:, :, ql:ql + 128],
                                tri_sb[:, :].rearrange("p x -> p () x")
                                .broadcast_to((128, 2, 128)))
                        if debug and qb == 1 and c == 2:
                            nc.sync.dma_start(
                                dPT[:, :],
                                pt[:, :, :].rearrange("p h n -> p (h n)"))
                        pts.append((c, ql, pt))
                    stage.append(pts)
                    if len(stage) == 4:
                        _emit_y(nc, v_sb, py, stage.pop(0), ntc)
                while stage:
                    _emit_y(nc, v_sb, py, stage.pop(0), ntc)

                # ---- denominators -> 1/den (cheap path) ----
                # dens ride in py[h] rows 64:96 (32 replicated rows). Stream-
                # transpose the [32,512] rider block so the q values spread
                # across lanes, reciprocal only lane-column 0 (16 els/lane
                # instead of 512), transpose back: den2 row 32h = 1/den_h.
                den2 = small.tile([64, 512], f32, tag="den2", name=f"d2{qb}")
                for h in range(2):
                    xs = small.tile([32, 512], f32, tag="xs",
                                    name=f"xs{qb}_{h}")
                    nc.vector.transpose(xs[0:32, :], py[h][64:96, :])
                    xv = xs[0:32, :].rearrange("p (j c) -> p j c", c=32)
                    nc.vector.reciprocal(xv[:, :, 0:1], xv[:, :, 0:1])
                    nc.vector.transpose(den2[32 * h:32 * h + 32, :],
                                        xs[0:32, :])
                rdenb = small.tile([64, 512], bf16, tag="rdenb",
                                   name=f"rb{qb}")
                nc.vector.tensor_copy(rdenb[0:33, :], den2[0:33, :])
                if debug:
                    nc.sync.dma_start(dDEN[:, q0:q0 + 512], rdenb[0:64, :])

                def fin_norm(qb=qb, q0=q0, py=py, rdenb=rdenb):
                    rp = pdn.tile([128, 512], f32, tag="dn", name=f"rp{qb}")
                    nc.tensor.matmul(rp[:, :], sel_sb[0:33, :],
                                     rdenb[0:33, :], start=True, stop=True)
                    rrs = tmp.tile([128, 512], bf16, tag="rrs",
                                   name=f"rrs{qb}")
                    nc.vector.tensor_copy(rrs[:, :], rp[:, :])
                    for h in range(2):
                        nc.vector.tensor_mul(
                            yT[64 * h:64 * h + 64, q0:q0 + 512],
                            py[h][0:64, :], rrs[64 * h:64 * h + 64, :])

                def mk_cproj(tt):
                    def f():
                        ts2 = slice(128 * tt, 128 * tt + 128)
                        for half in range(2):
                            po = pdn.tile([128, 512], f32, tag="dn",
                                          name=f"po{tt}_{half}")
                            nc.tensor.matmul(
                                po[:, :], yT[:, ts2],
                                wc_sb[:, 512 * half:512 * half + 512],
                                start=True, stop=True)
                            ob = obpool.tile([128, 512], bf16, tag="ob",
                                             name=f"ob{tt}_{half}")
                            nc.vector.tensor_copy(ob[:, :], po[:, :])
                            nc.sync.dma_start(
                                outp[ts2, 512 * half:512 * half + 512],
                                ob[:, :])
                    return f

                flush_all()
                pending.append(fin_norm)
                for tt in range(4 * qb, 4 * qb + 4):
                    pending.append(mk_cproj(tt))
            flush_all()
            if debug:
                nc.sync.dma_start(dQT[:, :], QT[:, :])
                nc.sync.dma_start(dKT[:, :], KT[:, :])
                nc.sync.dma_start(
                    dV[:, :], v_sb[:, :, :].rearrange("p tt d -> p (tt d)"))
                nc.sync.dma_start(dYT[:, :], yT[:, :])
    _cap_matmul_waits(nc)
    return nc


def _emit_y(nc, v_sb, py, pts, ntc):
    """PV accumulation with lhsT = [V_h(64) | ones(32)]: M=96, the ones
    columns accumulate the softmax denominator for free (rows 64:96)."""
    for (c, ql, pt) in pts:
        for h in range(2):
            nc.tensor.matmul(py[h][0:96, ql:512],
                             v_sb[:, c, 96 * h:96 * h + 96],
                             pt[:, h, ql:512],
                             start=(c == 0), stop=(c == ntc - 1))


def _cap_matmul_waits(nc, limit=1):
    """walrus supports few (often one) sync-wait slots per lowered
    instruction; move excess waits onto same-engine nops inserted just
    before, so the sequencer blocks identically but each instruction
    carries at most `limit` waits."""
    import bass_rust
    from concourse import mybir

    eng = {
        mybir.EngineType.PE: nc.tensor,
        mybir.EngineType.DVE: nc.vector,
        mybir.EngineType.Activation: nc.scalar,
        mybir.EngineType.Pool: nc.gpsimd,
        mybir.EngineType.SP: nc.sync,
    }

    def make_nop(e):
        eng[e].nop()
        fn = nc.m.functions[0]
        for obb in fn.blocks:
            if (obb.instructions
                    and type(obb.instructions[-1]).__name__ == 'InstNoOp'):
                return obb.instructions.pop()
        raise AssertionError('nop not found')

    fn = nc.m.functions[0]
    for bb in fn.blocks:
        il = bb.instructions
        i = 0
        while i < len(il):
            inst = il[i]
            si = inst.sync_info
            if (si and si.on_wait and len(si.on_wait) > limit
                    and inst.engine in eng
                    and type(inst).__name__ != 'InstNoOp'):
                waits = list(si.on_wait)
                keep, excess = waits[-limit:], waits[:-limit]
                for w in excess:
                    nop = make_nop(inst.engine)
                    nop.sync_info = bass_rust.SyncInfo(on_wait=[w],
                                                       on_update=[])
                    il.insert(i, nop)
                    i += 1
                inst.sync_info = bass_rust.SyncInfo(
                    on_wait=keep, on_update=list(si.on_update))
            i += 1


def _host_prep(x, ve, qkv_w, lambdas, c_proj_w, T):
    import ml_dtypes
    bf = ml_dtypes.bfloat16
    xT = np.ascontiguousarray(x.reshape(T, DIM).T.astype(bf))
    af = (1.0 / 1024.0) ** np.linspace(0.0, 1.0, HD // 4, dtype=np.float32)
    theta = np.arange(T, dtype=np.float32)[:, None] * af[None, :]
    cos = np.ascontiguousarray(np.cos(theta).astype(bf))
    sin = np.ascontiguousarray(np.sin(theta).astype(bf))
    tri = np.ascontiguousarray(np.triu(np.ones((128, 128), np.float32)).astype(bf))
    iden = np.ascontiguousarray(np.eye(128, dtype=np.float32).astype(bf))
    lam = np.asarray(lambdas, np.float32)
    vef = ve.reshape(T, DIM)
    in_maps = []
    for c in range(NC_CORES):
        sl = slice(128 * c, 128 * c + 128)
        wq = qkv_w[0][sl]
        wk = qkv_w[1][sl]
        wv = qkv_w[2][sl] * lam[0]
        wTl = np.ascontiguousarray(np.concatenate([wq, wk, wv], 0).T.astype(bf))
        ve_l = np.ones((T, 192), np.float32)
        ve_l[:, 0:64] = vef[:, sl.start:sl.start + 64] * lam[1]
        ve_l[:, 96:160] = vef[:, sl.start + 64:sl.stop] * lam[1]
        ve_l = np.ascontiguousarray(ve_l.astype(bf))
        wcTl = np.ascontiguousarray(c_proj_w[:, sl].T.astype(bf))
        selm = np.zeros((64, 128), np.float32)
        selm[0, 0:64] = 1.0
        selm[32, 64:128] = 1.0
        in_maps.append(dict(xT=xT, wT=wTl, ve=ve_l, cosn=cos, sinn=sin,
                            tri=tri, iden=iden, wcT=wcTl,
                            sel=np.ascontiguousarray(selm.astype(bf))))
    return in_maps


LAST_RESULTS = None


def kernel(x, ve, qkv_w, lambdas, c_proj_w):
    import sys
    if '/opt/trn_rl_repo' not in sys.path:
        sys.path.insert(0, '/opt/trn_rl_repo')
    from concourse.bass_utils import run_bass_kernel_spmd

    x = np.asarray(x, np.float32)
    T = x.shape[1]
    in_maps = _host_prep(np.asarray(x, np.float32), np.asarray(ve, np.float32),
                         np.asarray(qkv_w, np.float32),
                         np.asarray(lambdas, np.float32),
                         np.asarray(c_proj_w, np.float32), T)
    nc = _build(T)
    res = run_bass_kernel_spmd(nc, in_maps, core_ids=list(range(NC_CORES)))
    global LAST_RESULTS
    LAST_RESULTS = res
    out = np.zeros((T, DIM), np.float32)
    for rmap in res.results:
        out += rmap["outp"].astype(np.float32)
    return out.reshape(1, T, DIM)


# revision 59
# speedup vs baseline: 1.0423x; 1.0318x over previous
"""Trainium2 Bass kernel for a causal self-attention block (nanogpt-style).

Full inputs -> full output. 16 heads sharded 2/core across 8 NeuronCores
(tensor-parallel); each core computes its heads' QKV projection, per-head
RMSNorm + RoPE, causal no-max-softmax attention (scores bounded: q,k are
RMS-normalized so |s|<=64), and a partial c_proj over its 128-dim slice of
the residual. Host sums the 8 bf16 partial outputs in f32.

bf16 rewrite of the f32r baseline (650us -> ~297us measured):
  - all matmuls bf16: 1 cyc/row at any N (f32r measured ~2x slower on HW),
    FWL fast weight loads, half the DMA/SBUF traffic, bf16 output partials
  - exp fused across both heads per chunk: one ACTIVATE over a [128,2,N]
    view of a 2-bank PSUM score tile (144 calls instead of 288)
  - PV accumulation keeps the ones-rider (lhsT=[V_h|ones32], M=96) so the
    softmax denominator accumulates free in py rows 64:96; note start=True
    clears the ENTIRE psum bank, so regions sharing a bank must not carry
    independent start flags (this killed a col-packed variant)
  - 1/den via DVE 32x32 stream-transpose: spread q across lanes, divide only
    16 els/lane, transpose back (vs 55us of [1,512] reciprocals); one K=33
    indicator matmul (sel) replicates both heads' 1/den to 128 partitions
  - softmax-normalize fused into the psum->yT eviction multiply
  - deferred finalize: each q-block's rp/yT/c_proj ops are emitted inside
    the NEXT q-block's chunk loop so the den chain overlaps score matmuls
    (the in-order PE queue otherwise stalls ~3us per q-block)
  - y lags scores by a 3-pair software pipeline (pt pool 8 bufs)
"""

import numpy as np

DIM = 1024
NH = 16
HD = 64
SCALE = 0.12
NC_CORES = 8
HPC = NH // NC_CORES  # 2 heads per core


def _build(T=4096):
    import concourse.bass as bass
    import concourse.tile as tile
    from concourse import mybir

    f32 = mybir.dt.float32
    f32r = mybir.dt.float32r
    bf16 = mybir.dt.bfloat16
    AF = mybir.ActivationFunctionType

    NTT = T // 128   # 32 t-tiles of 128
    NQB = T // 512   # 8 q-blocks of 512
    NG = max(1, NTT // 8)     # norm/rope groups of 8 t-tiles
    GT = NTT // NG
    EPS = float(np.finfo(np.float32).eps)

    nc = bass.Bass("TRN2", target_bir_lowering=False, debug=False,
                   num_devices=NC_CORES)

    xT = nc.declare_dram_parameter("xT", [DIM, T], bf16, isOutput=False).ap()
    wT = nc.declare_dram_parameter("wT", [DIM, 384], bf16, isOutput=False).ap()
    ve = nc.declare_dram_parameter("ve", [T, 192], bf16, isOutput=False).ap()
    sel = nc.declare_dram_parameter("sel", [64, 128], bf16, isOutput=False).ap()
    cosn = nc.declare_dram_parameter("cosn", [T, 16], bf16, isOutput=False).ap()
    sinn = nc.declare_dram_parameter("sinn", [T, 16], bf16, isOutput=False).ap()
    tri = nc.declare_dram_parameter("tri", [128, 128], bf16, isOutput=False).ap()
    iden = nc.declare_dram_parameter("iden", [128, 128], bf16, isOutput=False).ap()
    wcT = nc.declare_dram_parameter("wcT", [128, DIM], bf16, isOutput=False).ap()
    outp = nc.declare_dram_parameter("outp", [T, DIM], bf16, isOutput=True).ap()
    import os
    debug = os.environ.get('KDEBUG', '0') == '1'
    if debug:
        dQT = nc.declare_dram_parameter("dQT", [128, T], bf16, isOutput=True).ap()
        dKT = nc.declare_dram_parameter("dKT", [128, T], bf16, isOutput=True).ap()
        dV = nc.declare_dram_parameter("dV", [128, NTT * 192], bf16, isOutput=True).ap()
        dYT = nc.declare_dram_parameter("dYT", [128, T], bf16, isOutput=True).ap()
        dDEN = nc.declare_dram_parameter("dDEN", [64, T], bf16, isOutput=True).ap()
        dPT = nc.declare_dram_parameter("dPT", [128, 1024], bf16, isOutput=True).ap()

    def r(ap):
        return ap.bitcast(f32r)

    with tile.TileContext(nc) as tc:
        with (
            tc.tile_pool(name="consts", bufs=1) as consts,
            tc.tile_pool(name="persist", bufs=1) as persist,
            tc.tile_pool(name="xstream", bufs=3) as xstream,
            tc.tile_pool(name="tmp", bufs=3) as tmp,
            tc.tile_pool(name="pt", bufs=8) as ptpool,
            tc.tile_pool(name="ob", bufs=3) as obpool,
            tc.tile_pool(name="small", bufs=4) as small,
            tc.tile_pool(name="sc", bufs=2, space="PSUM") as psc,      # 2x2 banks
            tc.tile_pool(name="py", bufs=3, space="PSUM") as pyp,      # 3 banks
            tc.tile_pool(name="dn", bufs=1, space="PSUM") as pdn,      # 1 bank
        ):
            # ---- constants ----
            tri_sb = consts.tile([128, 128], bf16, tag="tri")
            nc.sync.dma_start(tri_sb[:, :], tri)
            id_sb = consts.tile([128, 128], bf16, tag="iden")
            nc.sync.dma_start(id_sb[:, :], iden)
            cos_sb = consts.tile([128, NTT, 16], bf16, tag="cos")
            nc.sync.dma_start(cos_sb[:, :, :],
                              cosn.rearrange("(tt p) i -> p tt i", p=128))
            sin_sb = consts.tile([128, NTT, 16], bf16, tag="sin")
            nc.sync.dma_start(sin_sb[:, :, :],
                              sinn.rearrange("(tt p) i -> p tt i", p=128))
            w_sb = consts.tile([128, 8, 384], bf16, tag="w")
            nc.sync.dma_start(w_sb[:, :, :],
                              wT.rearrange("(dc p) e -> p dc e", p=128))
            wc_sb = consts.tile([128, DIM], bf16, tag="wc")
            nc.sync.dma_start(wc_sb[:, :], wcT)
            eps_sb = consts.tile([128, 1], f32, tag="eps")
            nc.gpsimd.memset(eps_sb[:, :], EPS)
            sel_sb = consts.tile([64, 128], bf16, tag="sel")
            nc.sync.dma_start(sel_sb[:, :], sel)

            # den staging: per head a [32,512] scratch for the stream-transpose
            # reciprocal trick; den2 rows {0,32} end up holding 1/den.

            q_nat = persist.tile([128, NTT, 128], bf16, tag="qnat")
            k_nat = persist.tile([128, NTT, 128], bf16, tag="knat")
            # rider layout: v0 | ones | v1 | ones (denominator rides as
            # redundant ones columns in the M=96 PV matmul)
            v_sb = persist.tile([128, NTT, 192], bf16, tag="v")
            QT = persist.tile([128, T], bf16, tag="QT")
            KT = persist.tile([128, T], bf16, tag="KT")
            yT = persist.tile([128, T], bf16, tag="yT")

            # v preloaded with lambda1 * ve slice (host-scaled)
            nc.sync.dma_start(v_sb[:, :, :],
                              ve.rearrange("(tt p) d -> p tt d", p=128))

            # ---- phases 1-3, pipelined per tile-group of GT=8 ----
            # emit group g's QKV (PE-heavy), then its norm/rope (DVE), then
            # the PREVIOUS group's transposes: the transposes' psum->QT casts
            # then drain while the next group's QKV matmuls keep the PE busy,
            # instead of crawling at DVE-cast rate after all of phase 2.
            def emit_p1(tt):
                xt = xstream.tile([128, 8, 128], bf16, tag="xt",
                                  name=f"xt{tt}")
                nc.sync.dma_start(
                    xt[:, :, :],
                    xT[:, 128 * tt:128 * tt + 128]
                    .rearrange("(dc p) t -> p dc t", p=128))
                ps = psc.tile([128, 1024], f32, tag="sc", name=f"ps1_{tt}")
                for dc in range(8):
                    nc.tensor.matmul(ps[:, 0:384], xt[:, dc, :], w_sb[:, dc, :],
                                     start=(dc == 0), stop=(dc == 7))
                nc.vector.tensor_copy(q_nat[:, tt, :], ps[:, 0:128])
                nc.vector.tensor_copy(k_nat[:, tt, :], ps[:, 128:256])
                nc.vector.tensor_add(v_sb[:, tt, 0:64], ps[:, 256:320],
                                     v_sb[:, tt, 0:64])
                nc.vector.tensor_add(v_sb[:, tt, 96:160], ps[:, 320:384],
                                     v_sb[:, tt, 96:160])

            def emit_p3(tts):
                for tt in tts:
                    for ni, (nat, dstT) in enumerate(((q_nat, QT),
                                                      (k_nat, KT))):
                        pt_ps = pyp.tile([128, 128], bf16, tag="py",
                                         name=f"tp{tt}_{ni}")
                        nc.tensor.transpose(pt_ps[:, :], nat[:, tt, :],
                                            id_sb[:, :])
                        nc.vector.tensor_copy(
                            dstT[:, 128 * tt:128 * tt + 128], pt_ps[:, :])

            for tt in range(NTT):
                emit_p1(tt)
            for g in range(NG):
                gsl = slice(GT * g, GT * g + GT)
                for nat in (q_nat, k_nat):
                    xg = nat[:, gsl, :]                       # [128, GT, 128]
                    xg4 = nat[:, gsl, :].rearrange("p a (h d) -> p a h d", h=2)
                    sq = tmp.tile([128, GT * 128], bf16, tag="sq")
                    nc.vector.tensor_mul(sq[:, :], xg, xg)
                    ssum = small.tile([128, GT, 2], f32, tag="ssum")
                    nc.vector.reduce_sum(
                        ssum[:, :, :],
                        sq[:, :].rearrange("p (a h d) -> p a h d", a=GT, h=2),
                        axis=mybir.AxisListType.X)
                    sstd = small.tile([128, GT, 2], f32, tag="sstd")
                    nc.scalar.activation(sstd[:, :, :], ssum[:, :, :],
                                         AF.Sqrt, bias=eps_sb[:, :],
                                         scale=1.0 / HD)
                    rinv = small.tile([128, GT, 2], f32, tag="rinv")
                    nc.vector.reciprocal(rinv[:, :, :], sstd[:, :, :])
                    rinvb = small.tile([128, GT, 2], bf16, tag="rinvb")
                    nc.vector.tensor_copy(rinvb[:, :, :], rinv[:, :, :])
                    nc.vector.tensor_mul(
                        xg4, xg4,
                        rinvb[:, :, :].broadcast_to((128, GT, 2, HD)))
                    # rope on pairs (d, d+32), d in [0,16)
                    x1 = nat[:, gsl, :].rearrange("p a (h d) -> p a h d", h=2)[:, :, :, 0:16]
                    x2 = nat[:, gsl, :].rearrange("p a (h d) -> p a h d", h=2)[:, :, :, 32:48]
                    cg = (cos_sb[:, gsl, :].rearrange("p a i -> p a () i")
                          .broadcast_to((128, GT, 2, 16)))
                    sg = (sin_sb[:, gsl, :].rearrange("p a i -> p a () i")
                          .broadcast_to((128, GT, 2, 16)))
                    t1 = tmp.tile([128, GT, 2, 16], bf16, tag="t1")
                    t2 = tmp.tile([128, GT, 2, 16], bf16, tag="t2")
                    t3 = tmp.tile([128, GT, 2, 16], bf16, tag="t3")
                    t4 = tmp.tile([128, GT, 2, 16], bf16, tag="t4")
                    nc.vector.tensor_mul(t1[:, :, :, :], x1, cg)
                    nc.vector.tensor_mul(t2[:, :, :, :], x2, sg)
                    nc.vector.tensor_mul(t3[:, :, :, :], x1, sg)
                    nc.vector.tensor_mul(t4[:, :, :, :], x2, cg)
                    nc.vector.tensor_add(x1, t1[:, :, :, :], t2[:, :, :, :])
                    nc.vector.tensor_sub(x2, t4[:, :, :, :], t3[:, :, :, :])
            emit_p3(range(NTT))

            # ---- phase 4: causal attention per q-block of 512 ----
            # deferred finalize of the previous q-block, split into small
            # closures drained one per chunk-pair so the in-order PE queue
            # never piles up behind the DVE den chain or c_proj evictions
            pending = []

            def flush_one():
                if pending:
                    pending.pop(0)()

            def flush_all():
                while pending:
                    pending.pop(0)()

            for qb in range(NQB):
                q0 = 512 * qb
                ntc = 4 * (qb + 1)
                py = [pyp.tile([128, 512], f32, tag="py", name=f"py{qb}_{h}")
                      for h in range(2)]
                stage = []  # software pipeline: y lags scores by three pairs
                for pi in range(ntc // 2):
                    if pi == 1:
                        flush_all()
                    pts = []
                    for c in (2 * pi, 2 * pi + 1):
                        p = c - 4 * qb
                        ql = max(0, 128 * p)
                        ts = slice(128 * c, 128 * c + 128)
                        s = psc.tile([128, 1024], f32, tag="sc",
                                     name=f"s{qb}_{c}")
                        for h in range(2):
                            hp = slice(64 * h, 64 * h + 64)
                            nc.tensor.matmul(
                                s[:, 512 * h + ql:512 * h + 512],
                                KT[hp, ts], QT[hp, q0 + ql:q0 + 512],
                                start=True, stop=True,
                                tile_position=(64 * h, 0))
                        pt = ptpool.tile([128, 2, 512], bf16, tag="pt",
                                         name=f"ptile{qb}_{c}")
                        nc.scalar.activation(
                            pt[:, :, ql:512],
                            s[:, :].rearrange("p (h n) -> p h n", h=2)[:, :, ql:512],
                            AF.Exp, scale=SCALE)
                        if p >= 0:
                            nc.vector.tensor_mul(
                                pt[:, :, ql:ql + 128], pt[:, :, ql:ql + 128],
                                tri_sb[:, :].rearrange("p x -> p () x")
                                .broadcast_to((128, 2, 128)))
                        if debug and qb == 1 and c == 2:
                            nc.sync.dma_start(
                                dPT[:, :],
                                pt[:, :, :].rearrange("p h n -> p (h n)"))
                        pts.append((c, ql, pt))
                    stage.append(pts)
                    if len(stage) == 4:
                        _emit_y(nc, v_sb, py, stage.pop(0), ntc)
                while stage:
                    _emit_y(nc, v_sb, py, stage.pop(0), ntc)

                # ---- denominators -> 1/den (cheap path) ----
                # dens ride in py[h] rows 64:96 (32 replicated rows). Stream-
                # transpose the [32,512] rider block so the q values spread
                # across lanes, reciprocal only lane-column 0 (16 els/lane
                # instead of 512), transpose back: den2 row 32h = 1/den_h.
                den2 = small.tile([64, 512], f32, tag="den2", name=f"d2{qb}")
                for h in range(2):
                    xs = small.tile([32, 512], f32, tag="xs",
                                    name=f"xs{qb}_{h}")
                    nc.vector.transpose(xs[0:32, :], py[h][64:96, :])
                    xv = xs[0:32, :].rearrange("p (j c) -> p j c", c=32)
                    nc.vector.reciprocal(xv[:, :, 0:1], xv[:, :, 0:1])
                    nc.vector.transpose(den2[32 * h:32 * h + 32, :],
                                        xs[0:32, :])
                rdenb = small.tile([64, 512], bf16, tag="rdenb",
                                   name=f"rb{qb}")
                nc.vector.tensor_copy(rdenb[0:33, :], den2[0:33, :])
                if debug:
                    nc.sync.dma_start(dDEN[:, q0:q0 + 512], rdenb[0:64, :])

                def fin_norm(qb=qb, q0=q0, py=py, rdenb=rdenb):
                    rp = pdn.tile([128, 512], f32, tag="dn", name=f"rp{qb}")
                    nc.tensor.matmul(rp[:, :], sel_sb[0:33, :],
                                     rdenb[0:33, :], start=True, stop=True)
                    rrs = tmp.tile([128, 512], bf16, tag="rrs",
                                   name=f"rrs{qb}")
                    nc.vector.tensor_copy(rrs[:, :], rp[:, :])
                    for h in range(2):
                        nc.vector.tensor_mul(
                            yT[64 * h:64 * h + 64, q0:q0 + 512],
                            py[h][0:64, :], rrs[64 * h:64 * h + 64, :])

                def mk_cproj(tt):
                    def f():
                        ts2 = slice(128 * tt, 128 * tt + 128)
                        for half in range(2):
                            po = pdn.tile([128, 512], f32, tag="dn",
                                          name=f"po{tt}_{half}")
                            nc.tensor.matmul(
                                po[:, :], yT[:, ts2],
                                wc_sb[:, 512 * half:512 * half + 512],
                                start=True, stop=True)
                            ob = obpool.tile([128, 512], bf16, tag="ob",
                                             name=f"ob{tt}_{half}")
                            nc.vector.tensor_copy(ob[:, :], po[:, :])
                            nc.sync.dma_start(
                                outp[ts2, 512 * half:512 * half + 512],
                                ob[:, :])
                    return f

                flush_all()
                pending.append(fin_norm)
                for tt in range(4 * qb, 4 * qb + 4):
                    pending.append(mk_cproj(tt))
            flush_all()
            if debug:
                nc.sync.dma_start(dQT[:, :], QT[:, :])
                nc.sync.dma_start(dKT[:, :], KT[:, :])
                nc.sync.dma_start(
                    dV[:, :], v_sb[:, :, :].rearrange("p tt d -> p (tt d)"))
                nc.sync.dma_start(dYT[:, :], yT[:, :])
    _cap_matmul_waits(nc)
    return nc


def _emit_y(nc, v_sb, py, pts, ntc):
    """PV accumulation with lhsT = [V_h(64) | ones(32)]: M=96, the ones
    columns accumulate the softmax denominator for free (rows 64:96)."""
    for (c, ql, pt) in pts:
        for h in range(2):
            nc.tensor.matmul(py[h][0:96, ql:512],
                             v_sb[:, c, 96 * h:96 * h + 96],
                             pt[:, h, ql:512],
                             start=(c == 0), stop=(c == ntc - 1))


def _cap_matmul_waits(nc, limit=1):
    """walrus supports few (often one) sync-wait slots per lowered
    instruction; move excess waits onto same-engine nops inserted just
    before, so the sequencer blocks identically but each instruction
    carries at most `limit` waits."""
    import bass_rust
    from concourse import mybir

    eng = {
        mybir.EngineType.PE: nc.tensor,
        mybir.EngineType.DVE: nc.vector,
        mybir.EngineType.Activation: nc.scalar,
        mybir.EngineType.Pool: nc.gpsimd,
        mybir.EngineType.SP: nc.sync,
    }

    def make_nop(e):
        eng[e].nop()
        fn = nc.m.functions[0]
        for obb in fn.blocks:
            if (obb.instructions
                    and type(obb.instructions[-1]).__name__ == 'InstNoOp'):
                return obb.instructions.pop()
        raise AssertionError('nop not found')

    fn = nc.m.functions[0]
    for bb in fn.blocks:
        il = bb.instructions
        i = 0
        while i < len(il):
            inst = il[i]
            si = inst.sync_info
            if (si and si.on_wait and len(si.on_wait) > limit
                    and inst.engine in eng
                    and type(inst).__name__ != 'InstNoOp'):
                waits = list(si.on_wait)
                keep, excess = waits[-limit:], waits[:-limit]
                for w in excess:
                    nop = make_nop(inst.engine)
                    nop.sync_info = bass_rust.SyncInfo(on_wait=[w],
                                                       on_update=[])
                    il.insert(i, nop)
                    i += 1
                inst.sync_info = bass_rust.SyncInfo(
                    on_wait=keep, on_update=list(si.on_update))
            i += 1


def _host_prep(x, ve, qkv_w, lambdas, c_proj_w, T):
    import ml_dtypes
    bf = ml_dtypes.bfloat16
    xT = np.ascontiguousarray(x.reshape(T, DIM).T.astype(bf))
    af = (1.0 / 1024.0) ** np.linspace(0.0, 1.0, HD // 4, dtype=np.float32)
    theta = np.arange(T, dtype=np.float32)[:, None] * af[None, :]
    cos = np.ascontiguousarray(np.cos(theta).astype(bf))
    sin = np.ascontiguousarray(np.sin(theta).astype(bf))
    tri = np.ascontiguousarray(np.triu(np.ones((128, 128), np.float32)).astype(bf))
    iden = np.ascontiguousarray(np.eye(128, dtype=np.float32).astype(bf))
    lam = np.asarray(lambdas, np.float32)
    vef = ve.reshape(T, DIM)
    in_maps = []
    for c in range(NC_CORES):
        sl = slice(128 * c, 128 * c + 128)
        wq = qkv_w[0][sl]
        wk = qkv_w[1][sl]
        wv = qkv_w[2][sl] * lam[0]
        wTl = np.ascontiguousarray(np.concatenate([wq, wk, wv], 0).T.astype(bf))
        ve_l = np.ones((T, 192), np.float32)
        ve_l[:, 0:64] = vef[:, sl.start:sl.start + 64] * lam[1]
        ve_l[:, 96:160] = vef[:, sl.start + 64:sl.stop] * lam[1]
        ve_l = np.ascontiguousarray(ve_l.astype(bf))
        wcTl = np.ascontiguousarray(c_proj_w[:, sl].T.astype(bf))
        selm = np.zeros((64, 128), np.float32)
        selm[0, 0:64] = 1.0
        selm[32, 64:128] = 1.0
        in_maps.append(dict(xT=xT, wT=wTl, ve=ve_l, cosn=cos, sinn=sin,
                            tri=tri, iden=iden, wcT=wcTl,
                            sel=np.ascontiguousarray(selm.astype(bf))))
    return in_maps


LAST_RESULTS = None


def kernel(x, ve, qkv_w, lambdas, c_proj_w):
    import sys
    if '/opt/trn_rl_repo' not in sys.path:
        sys.path.insert(0, '/opt/trn_rl_repo')
    from concourse.bass_utils import run_bass_kernel_spmd

    x = np.asarray(x, np.float32)
    T = x.shape[1]
    in_maps = _host_prep(np.asarray(x, np.float32), np.asarray(ve, np.float32),
                         np.asarray(qkv_w, np.float32),
                         np.asarray(lambdas, np.float32),
                         np.asarray(c_proj_w, np.float32), T)
    nc = _build(T)
    res = run_bass_kernel_spmd(nc, in_maps, core_ids=list(range(NC_CORES)))
    global LAST_RESULTS
    LAST_RESULTS = res
    out = np.zeros((T, DIM), np.float32)
    for rmap in res.results:
        out += rmap["outp"].astype(np.float32)
    return out.reshape(1, T, DIM)
